# revision 6
# baseline (speedup 1.0000x reference)
"""Trainium2 Bass kernel for nn_ConditionedDense (hypernetwork-conditioned dense).

Reference computation:
    A = einsum('bnp,pq->bnq', P, Wk)         # hypernetwork: per-position weights
    W = relu(A).reshape(B, N, c_in, c_out)
    out = einsum('bni,bnio->bno', X, W)

Strategy: pure data parallel over 8 NeuronCores (shard batch dim). Per core
16384 positions, tiled 128 positions/tile, 4 tiles per DMA chunk:
  - PE matmul computes A-tile [128 pos, 1024] in PSUM (lhsT = P^T tile,
    rhs = Wk, both bf16; Wk host-permuted to q = o*32+i layout)
  - ACT applies relu (PSUM -> SBUF, bf16 out)
  - DVE (and GPSIMD for a fraction of tiles) multiplies by X broadcast
    over o; DVE grouped-reduces over i (innermost) and upcasts to fp32
Host side (free): P transposed per shard, Wk column-permuted, X/P/Wk cast
to bf16.
"""

import os
from contextlib import ExitStack

import numpy as np
import ml_dtypes

import concourse.bass as bass
import concourse.tile as tile
from concourse import bacc, mybir
from concourse.bass_utils import run_bass_kernel_spmd

C_IN = 32
C_OUT = 32
P_DIM = 64
Q = C_IN * C_OUT  # 1024
B, N = 32, 4096
N_CORES = 8
B_SH = B // N_CORES          # 4 batches per core
NPOS = B_SH * N              # 16384 positions per core
TILE_P = 128                 # positions per tile
N_TILES = NPOS // TILE_P     # 128
CHUNK = 4                    # tiles per DMA chunk
N_CHUNKS = N_TILES // CHUNK  # 32

F32 = mybir.dt.float32
BF16 = mybir.dt.bfloat16

# Out of every 8 tiles, this many route their multiply to GPSIMD.
GS_OF_8 = int(os.environ.get("BASS_GS_OF_8", "0"))

_BUILD_CACHE = {}
LAST_RESULTS = None  # BassKernelResults of the most recent run (for profiling)


def _build_nc():
    nc = bacc.Bacc(
        "TRN2", target_bir_lowering=False, debug=False, num_devices=N_CORES
    )
    X_d = nc.declare_dram_parameter("X", [NPOS, C_IN], BF16, isOutput=False)
    PT_d = nc.declare_dram_parameter("PT", [P_DIM, NPOS], BF16, isOutput=False)
    Wk_d = nc.declare_dram_parameter("Wk", [P_DIM, Q], BF16, isOutput=False)
    out_d = nc.declare_dram_parameter("out", [NPOS, C_OUT], F32, isOutput=True)

    relu = mybir.ActivationFunctionType.Relu
    mult = mybir.AluOpType.mult
    add = mybir.AluOpType.add

    with ExitStack() as ctx:
        tc = ctx.enter_context(tile.TileContext(nc))
        wkp = ctx.enter_context(tc.tile_pool(name="wk", bufs=1))
        xp = ctx.enter_context(tc.tile_pool(name="x", bufs=3))
        pp = ctx.enter_context(tc.tile_pool(name="pT", bufs=3))
        apool = ctx.enter_context(tc.tile_pool(name="apsum", bufs=3, space="PSUM"))
        wp = ctx.enter_context(tc.tile_pool(name="w", bufs=4))
        mp = ctx.enter_context(tc.tile_pool(name="m", bufs=4))
        obp = ctx.enter_context(tc.tile_pool(name="ob", bufs=3))
        op = ctx.enter_context(tc.tile_pool(name="o", bufs=3))

        wk_t = wkp.tile([P_DIM, Q], BF16)
        nc.sync.dma_start(out=wk_t[:], in_=Wk_d[:])

        for ch in range(N_CHUNKS):
            # chunk loads: CHUNK * 128 positions per DMA
            x_c = xp.tile([TILE_P, CHUNK, C_IN], BF16)
            nc.sync.dma_start(
                out=x_c[:],
                in_=X_d[bass.ts(ch, TILE_P * CHUNK), :].rearrange(
                    "(a p) i -> p a i", p=TILE_P
                ),
            )
            pT_c = pp.tile([P_DIM, CHUNK * TILE_P], BF16)
            nc.sync.dma_start(
                out=pT_c[:], in_=PT_d[:, bass.ts(ch, TILE_P * CHUNK)]
            )
            o_c = op.tile([TILE_P, CHUNK, C_OUT], F32)

            for j in range(CHUNK):
                t = ch * CHUNK + j
                a_t = apool.tile([TILE_P, Q], F32)
                lhsT = pT_c[:, bass.ts(j, TILE_P)]
                nc.tensor.matmul(
                    a_t[:, 0:512], lhsT=lhsT, rhs=wk_t[:, 0:512],
                    start=True, stop=True,
                )
                nc.tensor.matmul(
                    a_t[:, 512:1024], lhsT=lhsT, rhs=wk_t[:, 512:1024],
                    start=True, stop=True,
                )

                # relu: PSUM -> SBUF, cast to bf16 (ACT engine)
                w_t = wp.tile([TILE_P, Q], BF16)
                nc.scalar.activation(w_t[:], a_t[:], relu)

                # m[p, o, i] = w[p, o, i] * x[p, i]
                m_t = mp.tile([TILE_P, Q], BF16)
                w3 = w_t[:].rearrange("p (o i) -> p o i", o=C_OUT)
                m3 = m_t[:].rearrange("p (o i) -> p o i", o=C_OUT)
                x3 = x_c[:, j, :].unsqueeze(1).broadcast_to(
                    [TILE_P, C_OUT, C_IN]
                )
                eng = nc.gpsimd if (t % 8) < GS_OF_8 else nc.vector
                eng.tensor_tensor(out=m3, in0=w3, in1=x3, op=mult)

                # grouped reduce over innermost i (DVE), bf16 out + upcast
                o_b = obp.tile([TILE_P, C_OUT], BF16)
                with nc.allow_low_precision("bf16 reduce, fp32 internal accum"):
                    nc.vector.tensor_reduce(
                        out=o_b[:], in_=m3, axis=mybir.AxisListType.X, op=add
                    )
                nc.vector.tensor_copy(out=o_c[:, j, :], in_=o_b[:])

            nc.sync.dma_start(
                out=out_d[bass.ts(ch, TILE_P * CHUNK), :].rearrange(
                    "(a p) i -> p a i", p=TILE_P
                ),
                in_=o_c[:],
            )

    nc.finalize()
    return nc


def _get_nc():
    key = "v2"
    if key not in _BUILD_CACHE:
        _BUILD_CACHE[key] = _build_nc()
    return _BUILD_CACHE[key]


def kernel(X, P, Wk):
    global LAST_RESULTS
    X = np.asarray(X, dtype=np.float32)
    P = np.asarray(P, dtype=np.float32)
    Wk = np.asarray(Wk, dtype=np.float32)
    bf16 = ml_dtypes.bfloat16

    # Host-side prep (free): shard, transpose P, permute Wk columns so the
    # device-side layout is q = o*32 + i; cast matmul operands to bf16.
    WkP = np.ascontiguousarray(
        Wk.reshape(P_DIM, C_IN, C_OUT).transpose(0, 2, 1).reshape(P_DIM, Q)
    ).astype(bf16)
    in_maps = []
    for c in range(N_CORES):
        Xc = np.ascontiguousarray(
            X[c * B_SH:(c + 1) * B_SH].reshape(NPOS, C_IN)
        ).astype(bf16)
        PTc = np.ascontiguousarray(
            P[c * B_SH:(c + 1) * B_SH].reshape(NPOS, P_DIM).T
        ).astype(bf16)
        in_maps.append({"X": Xc, "PT": PTc, "Wk": WkP})

    nc = _get_nc()
    trace = os.environ.get("BASS_PROFILE", "0") == "1"
    res = run_bass_kernel_spmd(nc, in_maps, list(range(N_CORES)), trace=trace)
    LAST_RESULTS = res

    out = np.empty((B, N, C_OUT), dtype=np.float32)
    for c in range(N_CORES):
        out[c * B_SH:(c + 1) * B_SH] = np.asarray(res.results[c]["out"]).reshape(
            B_SH, N, C_OUT
        )
    return out


# revision 7
# speedup vs baseline: 1.1936x; 1.1936x over previous
"""Trainium2 Bass kernel for nn_ConditionedDense (hypernetwork-conditioned dense).

Reference computation:
    A = einsum('bnp,pq->bnq', P, Wk)         # hypernetwork: per-position weights
    W = relu(A).reshape(B, N, c_in, c_out)
    out = einsum('bni,bnio->bno', X, W)

Strategy: pure data parallel over 8 NeuronCores (shard batch dim). Per core
16384 positions, tiled 128 positions/tile, 4 tiles per DMA chunk:
  - PE matmul computes A-tile [128 pos, 1024] in PSUM (lhsT = P^T tile,
    rhs = Wk, both bf16; Wk host-permuted to q = o*32+i layout)
  - ACT applies relu (PSUM -> SBUF, bf16 out)
  - DVE (and GPSIMD for a fraction of tiles) multiplies by X broadcast
    over o; DVE grouped-reduces over i (innermost) and upcasts to fp32
Host side (free): P transposed per shard, Wk column-permuted, X/P/Wk cast
to bf16.
"""

import os
from contextlib import ExitStack

import numpy as np
import ml_dtypes

import concourse.bass as bass
import concourse.tile as tile
from concourse import bacc, mybir
from concourse.bass_utils import run_bass_kernel_spmd

C_IN = 32
C_OUT = 32
P_DIM = 64
Q = C_IN * C_OUT  # 1024
B, N = 32, 4096
N_CORES = 8
B_SH = B // N_CORES          # 4 batches per core
NPOS = B_SH * N              # 16384 positions per core
TILE_P = 128                 # positions per tile
N_TILES = NPOS // TILE_P     # 128
CHUNK = 4                    # tiles per DMA chunk
N_CHUNKS = N_TILES // CHUNK  # 32

F32 = mybir.dt.float32
BF16 = mybir.dt.bfloat16

# Out of every 8 tiles, this many route their multiply to GPSIMD.
GS_OF_8 = int(os.environ.get("BASS_GS_OF_8", "0"))

_BUILD_CACHE = {}
LAST_RESULTS = None  # BassKernelResults of the most recent run (for profiling)


def _build_nc():
    nc = bacc.Bacc(
        "TRN2", target_bir_lowering=False, debug=False, num_devices=N_CORES
    )
    X_d = nc.declare_dram_parameter("X", [NPOS, C_IN], BF16, isOutput=False)
    PT_d = nc.declare_dram_parameter("PT", [P_DIM, NPOS], BF16, isOutput=False)
    Wk_d = nc.declare_dram_parameter("Wk", [P_DIM, Q], BF16, isOutput=False)
    out_d = nc.declare_dram_parameter("out", [NPOS, C_OUT], F32, isOutput=True)

    relu = mybir.ActivationFunctionType.Relu
    mult = mybir.AluOpType.mult
    add = mybir.AluOpType.add

    with ExitStack() as ctx:
        tc = ctx.enter_context(tile.TileContext(nc))
        wkp = ctx.enter_context(tc.tile_pool(name="wk", bufs=1))
        xp = ctx.enter_context(tc.tile_pool(name="x", bufs=3))
        pp = ctx.enter_context(tc.tile_pool(name="pT", bufs=3))
        apool = ctx.enter_context(tc.tile_pool(name="apsum", bufs=3, space="PSUM"))
        wp = ctx.enter_context(tc.tile_pool(name="w", bufs=4))
        mp = ctx.enter_context(tc.tile_pool(name="m", bufs=4))
        obp = ctx.enter_context(tc.tile_pool(name="ob", bufs=3))
        op = ctx.enter_context(tc.tile_pool(name="o", bufs=3))

        wk_t = wkp.tile([P_DIM, Q], BF16)
        nc.sync.dma_start(out=wk_t[:], in_=Wk_d[:])

        for ch in range(N_CHUNKS):
            # chunk loads: CHUNK * 128 positions per DMA
            x_c = xp.tile([TILE_P, CHUNK, C_IN], BF16)
            nc.sync.dma_start(
                out=x_c[:],
                in_=X_d[bass.ts(ch, TILE_P * CHUNK), :].rearrange(
                    "(a p) i -> p a i", p=TILE_P
                ),
            )
            pT_c = pp.tile([P_DIM, CHUNK * TILE_P], BF16)
            nc.sync.dma_start(
                out=pT_c[:], in_=PT_d[:, bass.ts(ch, TILE_P * CHUNK)]
            )
            o_c = op.tile([TILE_P, CHUNK, C_OUT], F32)

            for j in range(CHUNK):
                t = ch * CHUNK + j
                a_t = apool.tile([TILE_P, Q], F32)
                lhsT = pT_c[:, bass.ts(j, TILE_P)]
                nc.tensor.matmul(
                    a_t[:, 0:512], lhsT=lhsT, rhs=wk_t[:, 0:512],
                    start=True, stop=True,
                )
                nc.tensor.matmul(
                    a_t[:, 512:1024], lhsT=lhsT, rhs=wk_t[:, 512:1024],
                    start=True, stop=True,
                )

                # relu: PSUM -> SBUF, cast to bf16 (ACT engine)
                w_t = wp.tile([TILE_P, Q], BF16)
                nc.scalar.activation(w_t[:], a_t[:], relu)

                # m[p, o, i] = w[p, o, i] * x[p, i]
                m_t = mp.tile([TILE_P, Q], BF16)
                w3 = w_t[:].rearrange("p (o i) -> p o i", o=C_OUT)
                m3 = m_t[:].rearrange("p (o i) -> p o i", o=C_OUT)
                x3 = x_c[:, j, :].unsqueeze(1).broadcast_to(
                    [TILE_P, C_OUT, C_IN]
                )
                eng = nc.gpsimd if (t % 8) < GS_OF_8 else nc.vector
                eng.tensor_tensor(out=m3, in0=w3, in1=x3, op=mult)

                # grouped reduce over innermost i (DVE), bf16 out + upcast
                o_b = obp.tile([TILE_P, C_OUT], BF16)
                with nc.allow_low_precision("bf16 reduce, fp32 internal accum"):
                    nc.vector.tensor_reduce(
                        out=o_b[:], in_=m3, axis=mybir.AxisListType.X, op=add
                    )
                nc.vector.tensor_copy(out=o_c[:, j, :], in_=o_b[:])

            nc.sync.dma_start(
                out=out_d[bass.ts(ch, TILE_P * CHUNK), :].rearrange(
                    "(a p) i -> p a i", p=TILE_P
                ),
                in_=o_c[:],
            )

    nc.finalize()
    return nc


def _get_nc():
    key = "v2"
    if key not in _BUILD_CACHE:
        _BUILD_CACHE[key] = _build_nc()
    return _BUILD_CACHE[key]


def kernel(X, P, Wk):
    global LAST_RESULTS
    X = np.asarray(X, dtype=np.float32)
    P = np.asarray(P, dtype=np.float32)
    Wk = np.asarray(Wk, dtype=np.float32)
    bf16 = ml_dtypes.bfloat16

    # Host-side prep (free): shard, transpose P, permute Wk columns so the
    # device-side layout is q = o*32 + i; cast matmul operands to bf16.
    WkP = np.ascontiguousarray(
        Wk.reshape(P_DIM, C_IN, C_OUT).transpose(0, 2, 1).reshape(P_DIM, Q)
    ).astype(bf16)
    in_maps = []
    for c in range(N_CORES):
        Xc = np.ascontiguousarray(
            X[c * B_SH:(c + 1) * B_SH].reshape(NPOS, C_IN)
        ).astype(bf16)
        PTc = np.ascontiguousarray(
            P[c * B_SH:(c + 1) * B_SH].reshape(NPOS, P_DIM).T
        ).astype(bf16)
        in_maps.append({"X": Xc, "PT": PTc, "Wk": WkP})

    nc = _get_nc()
    trace = os.environ.get("BASS_PROFILE", "0") == "1"
    kw = {}
    if os.environ.get("BASS_TMPDIR"):
        kw["tmpdir"] = os.environ["BASS_TMPDIR"]
    res = run_bass_kernel_spmd(
        nc, in_maps, list(range(N_CORES)), trace=trace, **kw
    )
    LAST_RESULTS = res

    out = np.empty((B, N, C_OUT), dtype=np.float32)
    for c in range(N_CORES):
        out[c * B_SH:(c + 1) * B_SH] = np.asarray(res.results[c]["out"]).reshape(
            B_SH, N, C_OUT
        )
    return out
